# revision 15
# baseline (speedup 1.0000x reference)
"""GQA attention kernel for 8 Trainium2 NeuronCores.

Sharding: 2-way data parallel over batch x 4-way tensor parallel over heads.
Each core handles one batch element and 8 q-heads (2 kv-heads). The o-proj
partial outputs are summed on the host (replaces the all-reduce).

Per-core layout strategy: everything is kept transposed ([feature, seq]) so
every matmul consumes operands directly with the contraction dim on SBUF
partitions and no on-device transposes of activations are needed:
  Q^T = Wq_s^T @ x^T         (lhsT = Wq_s tiles, rhs = x^T tiles)
  S^T[k,q] = K^T_tile^T @ Q^T (k on partitions -> softmax denom via matmul)
  Y^T[d,q] = V_aug^T @ exp(S^T)  (V augmented with a ones column gives the
                                  softmax denominator for free in row 64)
  O^T = Wo_s^T @ (Y^T / Z)
Causality at [128k x 512q] tile granularity with per-diagonal-tile q-range
trimming; residual triangles masked with gpsimd affine_select after exp.
Softmax denominators inverted with reciprocal_approx_fast and broadcast to
partitions with gpsimd partition_broadcast. Head pairs (j, j+4) share the
PE array via 64-row tiling.

Pipelining: V transposes ride inside the projection loop; rope for chunk n
and attention for q-chunk n-1 interleave (rope perm PSUM comes from the
S-matmul pool); o-proj tiles for chunk n are drained into the exp-bound
inner attention loop of chunk n+1 between the S and Y matmuls.
"""

import numpy as np

B, T, C, D = 2, 2048, 2048, 64
KT = 16          # contraction tiles over C
NCH = 4          # 512-wide chunks over T
NQ = 512
ROPE_BASE = 10000.0
LPERM = [0, 4, 1, 5, 2, 6, 3, 7]  # local head order: pair j = (j, j+4)

_CACHE = {}


def _build_nc():
    import concourse.bass as bass  # noqa: F401
    import concourse.mybir as mybir
    from concourse import bacc
    from concourse.tile import TileContext
    from concourse.masks import make_identity

    F32 = mybir.dt.float32
    F16 = mybir.dt.float16
    AF = mybir.ActivationFunctionType

    def fr(ap):
        return ap

    nc = bacc.Bacc(None, target_bir_lowering=False, debug=True)
    xT = nc.dram_tensor("xT", [C, T], F16, kind="ExternalInput")
    wq = nc.dram_tensor("wq", [C, 512], F16, kind="ExternalInput")
    wk = nc.dram_tensor("wk", [C, 128], F16, kind="ExternalInput")
    wv = nc.dram_tensor("wv", [C, 128], F16, kind="ExternalInput")
    wo = nc.dram_tensor("wo", [512, C], F16, kind="ExternalInput")
    cosf = nc.dram_tensor("cosf", [128, T], F16, kind="ExternalInput")
    sinf = nc.dram_tensor("sinf", [128, T], F16, kind="ExternalInput")
    perm = nc.dram_tensor("perm", [128, 128], F16, kind="ExternalInput")
    outT = nc.dram_tensor("outT", [C, T], F16, kind="ExternalOutput")

    with TileContext(nc) as tc:
        with (
            tc.tile_pool(name="const", bufs=1) as cpool,
            tc.tile_pool(name="big", bufs=1) as bpool,
        ):
            wq_sb = cpool.tile([128, KT * 512], F16, tag="wq")
            wk_sb = cpool.tile([128, KT * 128], F16, tag="wk")
            wv_sb = cpool.tile([128, KT * 128], F16, tag="wv")
            cos_sb = cpool.tile([128, T], F16, tag="cos")
            sin_sb = cpool.tile([128, T], F16, tag="sin")
            perm_sb = cpool.tile([128, 128], F16, tag="perm")
            ident = cpool.tile([128, 128], F16, tag="ident")
            wo_sb = cpool.tile([128, 4 * 2048], F16, tag="wo")

            # persistent transposed activations
            qt = [bpool.tile([128, T], F16, tag=f"qt{j}", name=f"qt{j}") for j in range(4)]
            kt_sb = bpool.tile([128, T], F16, tag="ktT")
            vnat = bpool.tile([128, KT * 130], F16, tag="vnat")

            ebias = cpool.tile([128, 1], F32, tag="ebias")
            nc.vector.memset(ebias[:], -8.0)
            # preload the Exp activation table while DMAs run
            escr = cpool.tile([128, 1], F16, tag="escr")
            nc.scalar.activation(escr[:], ebias[:], AF.Exp, scale=1.0, bias=0.0)

            xpool = cpool.parent.alloc_tile_pool(name="xs", bufs=4)
            vtpool = cpool.parent.alloc_tile_pool(name="vtt", bufs=1)
            with (
                tc.tile_pool(name="pps", bufs=1, space="PSUM") as ppool,
                tc.tile_pool(name="vps", bufs=2, space="PSUM") as vpool,
            ):
                vt_sb = vtpool.tile([128, T], F16, tag="vtT")

                # chunk-0 x first so the first projection can start ASAP
                xsb_all = []
                for n in range(NCH):
                    xsb = []
                    for half in range(2):
                        xh = xpool.tile([128, 8 * NQ], F16, tag="xsb",
                                        name=f"x_{n}_{half}")
                        xsb.append(xh)
                    xsb_all.append(xsb)

                def load_x(n, nslice=2):
                    nsl = slice(n * NQ, (n + 1) * NQ)
                    for half in range(2):
                        for qtr in range(nslice):
                            w = 8 // nslice
                            nc.sync.dma_start(
                                out=xsb_all[n][half][:].rearrange(
                                    "p (kt t) -> p kt t", kt=8)[:, w * qtr:w * qtr + w],
                                in_=xT[:, :].rearrange("(kt p) t -> p kt t", p=128)[
                                    :, half * 8 + w * qtr: half * 8 + w * qtr + w, nsl
                                ],
                            )

                load_x(0, nslice=8)
                for qtr in range(KT):
                    nc.sync.dma_start(
                        out=wq_sb[:].rearrange("p (kt m) -> p kt m", kt=KT)[
                            :, qtr:qtr + 1],
                        in_=wq[:, :].rearrange("(kt p) m -> p kt m", p=128)[
                            :, qtr:qtr + 1],
                    )
                nc.sync.dma_start(
                    out=wk_sb[:].rearrange("p (kt m) -> p kt m", kt=KT),
                    in_=wk[:, :].rearrange("(kt p) m -> p kt m", p=128),
                )
                nc.sync.dma_start(
                    out=wv_sb[:].rearrange("p (kt m) -> p kt m", kt=KT),
                    in_=wv[:, :].rearrange("(kt p) m -> p kt m", p=128),
                )
                load_x(1)
                nc.sync.dma_start(out=cos_sb[:], in_=cosf[:, :])
                nc.sync.dma_start(out=sin_sb[:], in_=sinf[:, :])
                nc.sync.dma_start(out=perm_sb[:], in_=perm[:, :])
                for g in range(4):
                    nc.sync.dma_start(
                        out=wo_sb[:].rearrange("p (g m) -> p g m", g=4)[:, g:g + 1],
                        in_=wo[:, :].rearrange("(g p) m -> p g m", p=128)[:, g:g + 1],
                    )
                make_identity(nc, ident[:])
                nc.vector.memset(vnat[:], 1.0)
                # warm the PE clock gate while the first DMAs land
                for wi in range(24):
                    wq_ = vpool.tile([128, 128], F16, tag="vtps", name=f"warm_{wi}")
                    nc.tensor.transpose(wq_[:], ident[:], ident[:])

                # ---------------- projections + V layout ----------------
                # chunk 3 is deferred: its matmuls drain into the attention
                # phase as filler work for the exp-bound inner loops
                for n in range(NCH - 1):
                    if n == 2:
                        load_x(2)
                        load_x(3)
                    nsl = slice(n * NQ, (n + 1) * NQ)
                    xsb = xsb_all[n]
                    # m: 0-3 Q pairs, 4 K, 5 V
                    for m in range(6):
                        ps = ppool.tile([128, NQ], F32, tag=f"ps{m}")
                        for kt in range(KT):
                            if m < 4:
                                w_ap = wq_sb[:, kt * 512 + m * 128: kt * 512 + (m + 1) * 128]
                            elif m == 4:
                                w_ap = wk_sb[:, kt * 128:(kt + 1) * 128]
                            else:
                                w_ap = wv_sb[:, kt * 128:(kt + 1) * 128]
                            x_ap = xsb[kt // 8][:, (kt % 8) * NQ:(kt % 8 + 1) * NQ]
                            nc.tensor.matmul(
                                ps[:], fr(w_ap), fr(x_ap),
                                start=(kt == 0), stop=(kt == KT - 1),
                            )
                        dest = qt[m] if m < 4 else (kt_sb if m == 4 else vt_sb)
                        nc.scalar.copy(dest[:, nsl], ps[:])
                    # V natural-layout tiles for this chunk
                    for kt in range(4 * n, 4 * n + 4):
                        tp = vpool.tile([128, 128], F16, tag="vtps")
                        nc.tensor.transpose(tp[:], vt_sb[:, kt * 128:(kt + 1) * 128],
                                            ident[:])
                        nc.scalar.copy(
                            vnat[:, kt * 130: kt * 130 + 64], tp[:, 0:64])
                        nc.scalar.copy(
                            vnat[:, kt * 130 + 65: kt * 130 + 129], tp[:, 64:128])

            # ---------------- rope + attention + o-proj ----------------
            with (
                tc.tile_pool(name="sps", bufs=2, space="PSUM") as spool,
                tc.tile_pool(name="aps", bufs=2, space="PSUM") as apool,
                tc.tile_pool(name="ops", bufs=2, space="PSUM") as opool,
                tc.tile_pool(name="esb", bufs=3) as epool,
                tc.tile_pool(name="ysb", bufs=2) as ypool,
                tc.tile_pool(name="zsb", bufs=4) as zsbpool,
                tc.tile_pool(name="stg", bufs=3) as stpool,
                tc.tile_pool(name="rtmp", bufs=4) as rtpool,
            ):
                def emit_rope_tile(tile, n, ri):
                    nsl = slice(n * NQ, (n + 1) * NQ)
                    qs = spool.tile([128, 2 * NQ], F32, tag="sab",
                                    name=f"rope_{n}_{ri}")
                    # swap 32-halves within each 64 block (fp32 exact)
                    nc.tensor.matmul(qs[:, 0:NQ], perm_sb[:], tile[:, nsl],
                                     start=True, stop=True)
                    t1 = rtpool.tile([128, NQ], F16, tag="t1")
                    t2 = rtpool.tile([128, NQ], F16, tag="t2")
                    nc.vector.tensor_mul(t1[:], tile[:, nsl], cos_sb[:, nsl])
                    nc.vector.tensor_mul(t2[:], qs[:, 0:NQ], sin_sb[:, nsl])
                    nc.vector.tensor_add(tile[:, nsl], t1[:], t2[:])

                def emit_rope_chunk(n):
                    for ri, tile in enumerate([kt_sb, qt[0], qt[1], qt[2], qt[3]]):
                        emit_rope_tile(tile, n, ri)

                def make_proj3_thunks():
                    n3 = NCH - 1
                    nsl = slice(n3 * NQ, (n3 + 1) * NQ)
                    xsb = xsb_all[n3]
                    state = {}
                    thunks = []

                    def quarter(m, q):
                        if q == 0:
                            state[m] = opool.tile([128, NQ], F32, tag="opj",
                                                  name=f"p3ps{m}")
                        ps = state[m]
                        for kt in range(4 * q, 4 * q + 4):
                            if m < 4:
                                w_ap = wq_sb[:, kt * 512 + m * 128: kt * 512 + (m + 1) * 128]
                            elif m == 4:
                                w_ap = wk_sb[:, kt * 128:(kt + 1) * 128]
                            else:
                                w_ap = wv_sb[:, kt * 128:(kt + 1) * 128]
                            x_ap = xsb[kt // 8][:, (kt % 8) * NQ:(kt % 8 + 1) * NQ]
                            nc.tensor.matmul(
                                ps[:], fr(w_ap), fr(x_ap),
                                start=(kt == 0), stop=(kt == KT - 1),
                                skip_group_check=True,
                            )
                        if q == 3:
                            dest = qt[m] if m < 4 else (kt_sb if m == 4 else vt_sb)
                            nc.scalar.copy(dest[:, nsl], ps[:])

                    for m in range(6):
                        for q in range(4):
                            thunks.append(lambda m=m, q=q: quarter(m, q))

                    def vtrans3(kt):
                        tp = opool.tile([128, 256], F16, tag="opj", name=f"vt3_{kt}")
                        nc.tensor.transpose(tp[:, 0:128],
                                            vt_sb[:, kt * 128:(kt + 1) * 128], ident[:])
                        nc.scalar.copy(vnat[:, kt * 130: kt * 130 + 64], tp[:, 0:64])
                        nc.scalar.copy(vnat[:, kt * 130 + 65: kt * 130 + 129],
                                       tp[:, 64:128])

                    for kt in range(4 * n3, 4 * n3 + 4):
                        thunks.append(lambda kt=kt: vtrans3(kt))
                    return thunks

                tail_mode = {"on": False}

                def emit_oproj_tile(oqc, oyq, m):
                    oqsl = slice(oqc * NQ, (oqc + 1) * NQ)
                    ops = opool.tile([128, NQ], F32, tag="opj")
                    for g in range(4):
                        nc.tensor.matmul(
                            ops[:],
                            fr(wo_sb[:, g * 2048 + m * 128: g * 2048 + (m + 1) * 128]),
                            fr(oyq[g][:]), start=(g == 0), stop=(g == 3))
                    st = stpool.tile([128, NQ], F16, tag="st")
                    if tail_mode["on"]:
                        nc.scalar.copy(st[:], ops[:])
                    else:
                        nc.vector.tensor_copy(st[:], ops[:])
                    nc.sync.dma_start(
                        out=outT[m * 128:(m + 1) * 128, oqsl], in_=st[:])

                pending = make_proj3_thunks()
                for qc in range(NCH):
                    if qc == 0:
                        emit_rope_chunk(0)
                    qsl = slice(qc * NQ, (qc + 1) * NQ)
                    yq = [ypool.tile([128, NQ], F16, tag=f"yq{g}", name=f"yq{g}_{qc}") for g in range(4)]
                    kt_hi = 4 * (qc + 1)
                    niter = 4 * kt_hi
                    stride = max(1, niter // len(pending)) if pending else 1
                    it = 0
                    for pj in range(4):
                        yaugA = apool.tile([65, NQ], F32, tag="yaug", name=f"yaugA_{qc}_{pj}")
                        yaugB = apool.tile([65, NQ], F32, tag="yaug", name=f"yaugB_{qc}_{pj}")

                        def emit_y(kt, eab):
                            d = kt - 4 * qc
                            w0 = d * 128 if d > 0 else 0
                            nc.tensor.matmul(
                                yaugA[:, w0:NQ],
                                fr(vnat[:, kt * 130: kt * 130 + 65]),
                                fr(eab[:, w0:NQ]),
                                start=(kt == 0), stop=(kt == kt_hi - 1),
                                skip_group_check=True)
                            nc.tensor.matmul(
                                yaugB[:, w0:NQ],
                                fr(vnat[:, kt * 130 + 65: kt * 130 + 130]),
                                fr(eab[:, NQ + w0:2 * NQ]),
                                start=(kt == 0), stop=(kt == kt_hi - 1),
                                skip_group_check=True)

                        prev = None
                        for kt in range(kt_hi):
                            d = kt - 4 * qc
                            w0 = d * 128 if d > 0 else 0
                            nw = NQ - w0
                            sab = spool.tile([128, 2 * NQ], F32, tag="sab")
                            nc.tensor.matmul(
                                sab[:, w0:NQ],
                                fr(kt_sb[0:64, kt * 128:(kt + 1) * 128]),
                                fr(qt[pj][0:64, qc * NQ + w0:(qc + 1) * NQ]),
                                start=True, stop=True)
                            nc.tensor.matmul(
                                sab[:, NQ + w0:2 * NQ],
                                fr(kt_sb[64:128, kt * 128:(kt + 1) * 128]),
                                fr(qt[pj][64:128, qc * NQ + w0:(qc + 1) * NQ]),
                                start=True, stop=True)
                            # Y of the previous iteration: its exp finished an
                            # iteration ago, so it never stalls the PE queue
                            if prev is not None:
                                emit_y(*prev)
                            # drain one pending o-proj tile into the slack the
                            # exp on the scalar engine leaves on the PE array
                            it += 1
                            if pending and it % stride == 0:
                                pending.pop(0)()
                            eab = epool.tile([128, 2 * NQ], F16, tag="eab")
                            sab3 = sab[:].rearrange("p (h q) -> p h q", h=2)
                            eab3 = eab[:].rearrange("p (h q) -> p h q", h=2)
                            nc.scalar.activation(
                                eab3[:, :, w0:NQ], sab3[:, :, w0:NQ],
                                AF.Exp, scale=0.125, bias=ebias[:])
                            if d >= 0:
                                nc.gpsimd.affine_select(
                                    out=eab3[:, :, w0:NQ],
                                    in_=eab3[:, :, w0:NQ],
                                    compare_op=mybir.AluOpType.is_ge,
                                    fill=0.0,
                                    base=0,
                                    channel_multiplier=-1,
                                    pattern=[[0, 2], [1, nw]],
                                )
                            prev = (kt, eab)
                        emit_y(*prev)
                        # normalize: yq rows = numerator / Z
                        zcA = zsbpool.tile([1, NQ], F32, tag="zc", name=f"zcA_{qc}_{pj}")
                        zcB = zsbpool.tile([1, NQ], F32, tag="zc", name=f"zcB_{qc}_{pj}")
                        nc.vector.tensor_copy(zcA[0:1, :], yaugA[64:65, :])
                        nc.vector.tensor_copy(zcB[0:1, :], yaugB[64:65, :])
                        ziA = zsbpool.tile([1, NQ], F32, tag="zi", name=f"ziA_{qc}_{pj}")
                        ziB = zsbpool.tile([1, NQ], F32, tag="zi", name=f"ziB_{qc}_{pj}")
                        nc.vector.reciprocal_approx_fast(out=ziA[0:1, :], in_=zcA[0:1, :])
                        nc.vector.reciprocal_approx_fast(out=ziB[0:1, :], in_=zcB[0:1, :])
                        zbA = zsbpool.tile([64, NQ], F32, tag="zbA", name=f"zbA_{qc}_{pj}")
                        zbB = zsbpool.tile([64, NQ], F32, tag="zbB", name=f"zbB_{qc}_{pj}")
                        nc.gpsimd.partition_broadcast(zbA[0:64, :], ziA[0:1, :], channels=64)
                        nc.gpsimd.partition_broadcast(zbB[0:64, :], ziB[0:1, :], channels=64)
                        nc.vector.tensor_mul(yq[pj][0:64, :], yaugA[0:64, :], zbA[0:64, :])
                        nc.vector.tensor_mul(yq[pj][64:128, :], yaugB[0:64, :], zbB[0:64, :])
                        # rope the next chunk's tiles here so they are ready
                        # long before the next q-chunk's attention starts
                        if qc + 1 < NCH:
                            if pj == 0:
                                emit_rope_tile(kt_sb, qc + 1, 0)
                                emit_rope_tile(qt[0], qc + 1, 1)
                            else:
                                emit_rope_tile(qt[pj], qc + 1, 1 + pj)
                    # o-proj for this q chunk: enqueue; drained inside the next
                    # chunk's inner loop
                    for m in range(16):
                        pending.append(
                            lambda qc=qc, yq=yq, m=m: emit_oproj_tile(qc, yq, m))
                tail_mode["on"] = True
                while pending:
                    pending.pop(0)()
            vtpool.release()
            xpool.release()
    nc.finalize()
    return nc


def _rope_tables():
    inv = 1.0 / (ROPE_BASE ** (np.arange(0, D, 2, dtype=np.float32) / D))
    fr_ = np.arange(T, dtype=np.float32)[:, None] * inv[None, :]
    cosT = np.cos(fr_).T.astype(np.float32)
    sinT = np.sin(fr_).T.astype(np.float32)
    cosfull = np.ascontiguousarray(np.tile(cosT, (4, 1)))
    sinfull = np.ascontiguousarray(np.concatenate([-sinT, sinT, -sinT, sinT]))
    return cosfull, sinfull


def _perm_matrix():
    p = np.zeros((128, 128), dtype=np.float32)
    for i in range(128):
        j = i + 32 if (i % 64) < 32 else i - 32
        p[i, j] = 1.0
    return p


def _get_nc():
    if "nc" not in _CACHE:
        _CACHE["nc"] = _build_nc()
    return _CACHE["nc"]


def make_in_maps(x, Wq, Wk, Wv, Wo):
    cosfull, sinfull = _rope_tables()
    permm = _perm_matrix()
    in_maps = []
    for c in range(8):
        b, r = divmod(c, 4)
        qcols = np.concatenate(
            [np.arange(64 * (8 * r + h), 64 * (8 * r + h) + 64) for h in LPERM])
        in_maps.append({
            "xT": np.ascontiguousarray(x[b].T).astype(np.float16),
            "wq": np.ascontiguousarray(Wq[:, qcols]).astype(np.float16),
            "wk": np.ascontiguousarray(Wk[:, 128 * r:128 * (r + 1)]).astype(np.float16),
            "wv": np.ascontiguousarray(Wv[:, 128 * r:128 * (r + 1)]).astype(np.float16),
            "wo": np.ascontiguousarray(Wo[qcols, :]).astype(np.float16),
            "cosf": cosfull.astype(np.float16),
            "sinf": sinfull.astype(np.float16),
            "perm": permm.astype(np.float16),
        })
    return in_maps


def run(x, Wq, Wk, Wv, Wo, **spmd_kwargs):
    from concourse.bass_utils import run_bass_kernel_spmd

    nc = _get_nc()
    in_maps = make_in_maps(x, Wq, Wk, Wv, Wo)
    res = run_bass_kernel_spmd(nc, in_maps, list(range(8)), **spmd_kwargs)
    out = np.zeros((B, T, C), dtype=np.float32)
    for c in range(8):
        out[c // 4] += res.results[c]["outT"].T.astype(np.float32)
    return out, res


def kernel(**inputs):
    x = np.asarray(inputs["x"], dtype=np.float32)
    Wq = np.asarray(inputs["Wq"], dtype=np.float32)
    Wk = np.asarray(inputs["Wk"], dtype=np.float32)
    Wv = np.asarray(inputs["Wv"], dtype=np.float32)
    Wo = np.asarray(inputs["Wo"], dtype=np.float32)
    out, _ = run(x, Wq, Wk, Wv, Wo)
    return out


# revision 17
# speedup vs baseline: 1.0196x; 1.0196x over previous
"""GQA attention kernel for 8 Trainium2 NeuronCores.

Sharding: 2-way data parallel over batch x 4-way tensor parallel over heads.
Each core handles one batch element and 8 q-heads (2 kv-heads). The o-proj
partial outputs are summed on the host (replaces the all-reduce).

Per-core layout strategy: everything is kept transposed ([feature, seq]) so
every matmul consumes operands directly with the contraction dim on SBUF
partitions and no on-device transposes of activations are needed:
  Q^T = Wq_s^T @ x^T         (lhsT = Wq_s tiles, rhs = x^T tiles)
  S^T[k,q] = K^T_tile^T @ Q^T (k on partitions -> softmax denom via matmul)
  Y^T[d,q] = V_aug^T @ exp(S^T)  (V augmented with a ones column gives the
                                  softmax denominator for free in row 64)
  O^T = Wo_s^T @ (Y^T / Z)
Causality at [128k x 512q] tile granularity with per-diagonal-tile q-range
trimming; residual triangles masked with gpsimd affine_select after exp.
Softmax denominators inverted with reciprocal_approx_fast and broadcast to
partitions with gpsimd partition_broadcast. Head pairs (j, j+4) share the
PE array via 64-row tiling.

Pipelining: V transposes ride inside the projection loop; rope for chunk n
and attention for q-chunk n-1 interleave (rope perm PSUM comes from the
S-matmul pool); o-proj tiles for chunk n are drained into the exp-bound
inner attention loop of chunk n+1 between the S and Y matmuls.
"""

import numpy as np

B, T, C, D = 2, 2048, 2048, 64
KT = 16          # contraction tiles over C
NCH = 4          # 512-wide chunks over T
NQ = 512
ROPE_BASE = 10000.0
LPERM = [0, 4, 1, 5, 2, 6, 3, 7]  # local head order: pair j = (j, j+4)

_CACHE = {}


def _build_nc():
    import concourse.bass as bass  # noqa: F401
    import concourse.mybir as mybir
    from concourse import bacc
    from concourse.tile import TileContext
    from concourse.masks import make_identity

    F32 = mybir.dt.float32
    F16 = mybir.dt.float16
    AF = mybir.ActivationFunctionType

    def fr(ap):
        return ap

    nc = bacc.Bacc(None, target_bir_lowering=False, debug=True)
    xT = nc.dram_tensor("xT", [C, T], F16, kind="ExternalInput")
    wq = nc.dram_tensor("wq", [C, 512], F16, kind="ExternalInput")
    wk = nc.dram_tensor("wk", [C, 128], F16, kind="ExternalInput")
    wv = nc.dram_tensor("wv", [C, 128], F16, kind="ExternalInput")
    wo = nc.dram_tensor("wo", [512, C], F16, kind="ExternalInput")
    cosf = nc.dram_tensor("cosf", [128, T], F16, kind="ExternalInput")
    sinf = nc.dram_tensor("sinf", [128, T], F16, kind="ExternalInput")
    perm = nc.dram_tensor("perm", [128, 128], F16, kind="ExternalInput")
    outT = nc.dram_tensor("outT", [C, T], F16, kind="ExternalOutput")

    with TileContext(nc) as tc:
        with (
            tc.tile_pool(name="const", bufs=1) as cpool,
            tc.tile_pool(name="big", bufs=1) as bpool,
        ):
            wq_sb = cpool.tile([128, KT * 512], F16, tag="wq")
            wk_sb = cpool.tile([128, KT * 128], F16, tag="wk")
            wv_sb = cpool.tile([128, KT * 128], F16, tag="wv")
            cos_sb = cpool.tile([128, T], F16, tag="cos")
            sin_sb = cpool.tile([128, T], F16, tag="sin")
            perm_sb = cpool.tile([128, 128], F16, tag="perm")
            ident = cpool.tile([128, 128], F16, tag="ident")
            wo_sb = cpool.tile([128, 4 * 2048], F16, tag="wo")

            # persistent transposed activations
            qt = [bpool.tile([128, T], F16, tag=f"qt{j}", name=f"qt{j}") for j in range(4)]
            kt_sb = bpool.tile([128, T], F16, tag="ktT")
            vnat = bpool.tile([128, KT * 130], F16, tag="vnat")

            ebias = cpool.tile([128, 1], F32, tag="ebias")
            nc.vector.memset(ebias[:], -8.0)
            # preload the Exp activation table while DMAs run
            escr = cpool.tile([128, 1], F16, tag="escr")
            nc.scalar.activation(escr[:], ebias[:], AF.Exp, scale=1.0, bias=0.0)

            xpool = cpool.parent.alloc_tile_pool(name="xs", bufs=4)
            vtpool = cpool.parent.alloc_tile_pool(name="vtt", bufs=1)
            with (
                tc.tile_pool(name="pps", bufs=1, space="PSUM") as ppool,
                tc.tile_pool(name="vps", bufs=2, space="PSUM") as vpool,
            ):
                vt_sb = vtpool.tile([128, T], F16, tag="vtT")

                # chunk-0 x first so the first projection can start ASAP
                xsb_all = []
                for n in range(NCH):
                    xsb = []
                    for half in range(2):
                        xh = xpool.tile([128, 8 * NQ], F16, tag="xsb",
                                        name=f"x_{n}_{half}")
                        xsb.append(xh)
                    xsb_all.append(xsb)

                def load_x(n):
                    nsl = slice(n * NQ, (n + 1) * NQ)
                    for half in range(2):
                        for qtr in range(2):
                            nc.sync.dma_start(
                                out=xsb_all[n][half][:].rearrange(
                                    "p (kt t) -> p kt t", kt=8)[:, 4 * qtr:4 * qtr + 4],
                                in_=xT[:, :].rearrange("(kt p) t -> p kt t", p=128)[
                                    :, half * 8 + 4 * qtr: half * 8 + 4 * qtr + 4, nsl
                                ],
                            )

                # chunk-0 feeds the very first matmuls: emit its wq/x DMAs
                # per-kt in consumption order so the kt-outer first chunk can
                # start as soon as the first slices land
                for kt in range(KT):
                    nc.sync.dma_start(
                        out=wq_sb[:].rearrange("p (kt m) -> p kt m", kt=KT)[
                            :, kt:kt + 1],
                        in_=wq[:, :].rearrange("(kt p) m -> p kt m", p=128)[
                            :, kt:kt + 1],
                    )
                    nc.sync.dma_start(
                        out=xsb_all[0][kt // 8][:].rearrange(
                            "p (kt t) -> p kt t", kt=8)[:, kt % 8:kt % 8 + 1],
                        in_=xT[:, :].rearrange("(kt p) t -> p kt t", p=128)[
                            :, kt:kt + 1, 0:NQ],
                    )
                nc.sync.dma_start(
                    out=wk_sb[:].rearrange("p (kt m) -> p kt m", kt=KT),
                    in_=wk[:, :].rearrange("(kt p) m -> p kt m", p=128),
                )
                nc.sync.dma_start(
                    out=wv_sb[:].rearrange("p (kt m) -> p kt m", kt=KT),
                    in_=wv[:, :].rearrange("(kt p) m -> p kt m", p=128),
                )
                load_x(1)
                nc.sync.dma_start(out=cos_sb[:], in_=cosf[:, :])
                nc.sync.dma_start(out=sin_sb[:], in_=sinf[:, :])
                nc.sync.dma_start(out=perm_sb[:], in_=perm[:, :])
                for g in range(4):
                    nc.sync.dma_start(
                        out=wo_sb[:].rearrange("p (g m) -> p g m", g=4)[:, g:g + 1],
                        in_=wo[:, :].rearrange("(g p) m -> p g m", p=128)[:, g:g + 1],
                    )
                make_identity(nc, ident[:])
                nc.vector.memset(vnat[:], 1.0)
                # warm the PE clock gate while the first DMAs land
                for wi in range(24):
                    wq_ = vpool.tile([128, 128], F16, tag="vtps", name=f"warm_{wi}")
                    nc.tensor.transpose(wq_[:], ident[:], ident[:])

                # ---------------- projections + V layout ----------------
                # chunk 3 is deferred: its matmuls drain into the attention
                # phase as filler work for the exp-bound inner loops
                def proj_w_ap(m, kt):
                    if m < 4:
                        return wq_sb[:, kt * 512 + m * 128: kt * 512 + (m + 1) * 128]
                    if m == 4:
                        return wk_sb[:, kt * 128:(kt + 1) * 128]
                    return wv_sb[:, kt * 128:(kt + 1) * 128]

                for n in range(NCH - 1):
                    if n == 2:
                        load_x(2)
                        load_x(3)
                    nsl = slice(n * NQ, (n + 1) * NQ)
                    xsb = xsb_all[n]
                    # m: 0-3 Q pairs, 4 K, 5 V
                    if n == 0:
                        # kt-outer: consumption paced to DMA arrival order
                        pstiles = [ppool.tile([128, NQ], F32, tag=f"ps{m}",
                                               name=f"ps0_{m}")
                                   for m in range(6)]
                        for kt in range(KT):
                            x_ap = xsb[kt // 8][:, (kt % 8) * NQ:(kt % 8 + 1) * NQ]
                            for m in range(6):
                                nc.tensor.matmul(
                                    pstiles[m][:], fr(proj_w_ap(m, kt)), fr(x_ap),
                                    start=(kt == 0), stop=(kt == KT - 1),
                                    skip_group_check=True,
                                )
                        for m in range(6):
                            dest = qt[m] if m < 4 else (kt_sb if m == 4 else vt_sb)
                            nc.scalar.copy(dest[:, nsl], pstiles[m][:])
                    else:
                        for m in range(6):
                            ps = ppool.tile([128, NQ], F32, tag=f"ps{m}")
                            for kt in range(KT):
                                x_ap = xsb[kt // 8][:, (kt % 8) * NQ:(kt % 8 + 1) * NQ]
                                nc.tensor.matmul(
                                    ps[:], fr(proj_w_ap(m, kt)), fr(x_ap),
                                    start=(kt == 0), stop=(kt == KT - 1),
                                )
                            dest = qt[m] if m < 4 else (kt_sb if m == 4 else vt_sb)
                            nc.scalar.copy(dest[:, nsl], ps[:])
                    # V natural-layout tiles for this chunk
                    for kt in range(4 * n, 4 * n + 4):
                        tp = vpool.tile([128, 128], F16, tag="vtps")
                        nc.tensor.transpose(tp[:], vt_sb[:, kt * 128:(kt + 1) * 128],
                                            ident[:])
                        nc.scalar.copy(
                            vnat[:, kt * 130: kt * 130 + 64], tp[:, 0:64])
                        nc.scalar.copy(
                            vnat[:, kt * 130 + 65: kt * 130 + 129], tp[:, 64:128])

            # ---------------- rope + attention + o-proj ----------------
            with (
                tc.tile_pool(name="sps", bufs=2, space="PSUM") as spool,
                tc.tile_pool(name="aps", bufs=2, space="PSUM") as apool,
                tc.tile_pool(name="ops", bufs=2, space="PSUM") as opool,
                tc.tile_pool(name="esb", bufs=3) as epool,
                tc.tile_pool(name="ysb", bufs=2) as ypool,
                tc.tile_pool(name="zsb", bufs=4) as zsbpool,
                tc.tile_pool(name="stg", bufs=3) as stpool,
                tc.tile_pool(name="rtmp", bufs=4) as rtpool,
            ):
                def emit_rope_tile(tile, n, ri):
                    nsl = slice(n * NQ, (n + 1) * NQ)
                    qs = spool.tile([128, 2 * NQ], F32, tag="sab",
                                    name=f"rope_{n}_{ri}")
                    # swap 32-halves within each 64 block (fp32 exact)
                    nc.tensor.matmul(qs[:, 0:NQ], perm_sb[:], tile[:, nsl],
                                     start=True, stop=True)
                    t1 = rtpool.tile([128, NQ], F16, tag="t1")
                    t2 = rtpool.tile([128, NQ], F16, tag="t2")
                    nc.vector.tensor_mul(t1[:], tile[:, nsl], cos_sb[:, nsl])
                    nc.vector.tensor_mul(t2[:], qs[:, 0:NQ], sin_sb[:, nsl])
                    nc.vector.tensor_add(tile[:, nsl], t1[:], t2[:])

                def emit_rope_chunk(n):
                    for ri, tile in enumerate([kt_sb, qt[0], qt[1], qt[2], qt[3]]):
                        emit_rope_tile(tile, n, ri)

                def make_proj3_thunks():
                    n3 = NCH - 1
                    nsl = slice(n3 * NQ, (n3 + 1) * NQ)
                    xsb = xsb_all[n3]
                    state = {}
                    thunks = []

                    def quarter(m, q):
                        if q == 0:
                            state[m] = opool.tile([128, NQ], F32, tag="opj",
                                                  name=f"p3ps{m}")
                        ps = state[m]
                        for kt in range(4 * q, 4 * q + 4):
                            if m < 4:
                                w_ap = wq_sb[:, kt * 512 + m * 128: kt * 512 + (m + 1) * 128]
                            elif m == 4:
                                w_ap = wk_sb[:, kt * 128:(kt + 1) * 128]
                            else:
                                w_ap = wv_sb[:, kt * 128:(kt + 1) * 128]
                            x_ap = xsb[kt // 8][:, (kt % 8) * NQ:(kt % 8 + 1) * NQ]
                            nc.tensor.matmul(
                                ps[:], fr(w_ap), fr(x_ap),
                                start=(kt == 0), stop=(kt == KT - 1),
                                skip_group_check=True,
                            )
                        if q == 3:
                            dest = qt[m] if m < 4 else (kt_sb if m == 4 else vt_sb)
                            nc.scalar.copy(dest[:, nsl], ps[:])

                    for m in range(6):
                        for q in range(4):
                            thunks.append(lambda m=m, q=q: quarter(m, q))

                    def vtrans3(kt):
                        tp = opool.tile([128, 256], F16, tag="opj", name=f"vt3_{kt}")
                        nc.tensor.transpose(tp[:, 0:128],
                                            vt_sb[:, kt * 128:(kt + 1) * 128], ident[:])
                        nc.scalar.copy(vnat[:, kt * 130: kt * 130 + 64], tp[:, 0:64])
                        nc.scalar.copy(vnat[:, kt * 130 + 65: kt * 130 + 129],
                                       tp[:, 64:128])

                    for kt in range(4 * n3, 4 * n3 + 4):
                        thunks.append(lambda kt=kt: vtrans3(kt))
                    return thunks

                tail_mode = {"on": False}

                def emit_oproj_tile(oqc, oyq, m):
                    oqsl = slice(oqc * NQ, (oqc + 1) * NQ)
                    ops = opool.tile([128, NQ], F32, tag="opj")
                    for g in range(4):
                        nc.tensor.matmul(
                            ops[:],
                            fr(wo_sb[:, g * 2048 + m * 128: g * 2048 + (m + 1) * 128]),
                            fr(oyq[g][:]), start=(g == 0), stop=(g == 3))
                    st = stpool.tile([128, NQ], F16, tag="st")
                    if tail_mode["on"]:
                        nc.scalar.copy(st[:], ops[:])
                    else:
                        nc.vector.tensor_copy(st[:], ops[:])
                    nc.sync.dma_start(
                        out=outT[m * 128:(m + 1) * 128, oqsl], in_=st[:])

                pending = make_proj3_thunks()
                for qc in range(NCH):
                    if qc == 0:
                        emit_rope_chunk(0)
                    qsl = slice(qc * NQ, (qc + 1) * NQ)
                    yq = [ypool.tile([128, NQ], F16, tag=f"yq{g}", name=f"yq{g}_{qc}") for g in range(4)]
                    kt_hi = 4 * (qc + 1)
                    niter = 4 * kt_hi
                    stride = max(1, niter // len(pending)) if pending else 1
                    it = 0
                    for pj in range(4):
                        yaugA = apool.tile([65, NQ], F32, tag="yaug", name=f"yaugA_{qc}_{pj}")
                        yaugB = apool.tile([65, NQ], F32, tag="yaug", name=f"yaugB_{qc}_{pj}")

                        def emit_y(kt, eab):
                            d = kt - 4 * qc
                            w0 = d * 128 if d > 0 else 0
                            nc.tensor.matmul(
                                yaugA[:, w0:NQ],
                                fr(vnat[:, kt * 130: kt * 130 + 65]),
                                fr(eab[:, w0:NQ]),
                                start=(kt == 0), stop=(kt == kt_hi - 1),
                                skip_group_check=True)
                            nc.tensor.matmul(
                                yaugB[:, w0:NQ],
                                fr(vnat[:, kt * 130 + 65: kt * 130 + 130]),
                                fr(eab[:, NQ + w0:2 * NQ]),
                                start=(kt == 0), stop=(kt == kt_hi - 1),
                                skip_group_check=True)

                        prev = None
                        for kt in range(kt_hi):
                            d = kt - 4 * qc
                            w0 = d * 128 if d > 0 else 0
                            nw = NQ - w0
                            sab = spool.tile([128, 2 * NQ], F32, tag="sab")
                            nc.tensor.matmul(
                                sab[:, w0:NQ],
                                fr(kt_sb[0:64, kt * 128:(kt + 1) * 128]),
                                fr(qt[pj][0:64, qc * NQ + w0:(qc + 1) * NQ]),
                                start=True, stop=True)
                            nc.tensor.matmul(
                                sab[:, NQ + w0:2 * NQ],
                                fr(kt_sb[64:128, kt * 128:(kt + 1) * 128]),
                                fr(qt[pj][64:128, qc * NQ + w0:(qc + 1) * NQ]),
                                start=True, stop=True)
                            # Y of the previous iteration: its exp finished an
                            # iteration ago, so it never stalls the PE queue
                            if prev is not None:
                                emit_y(*prev)
                            # drain one pending o-proj tile into the slack the
                            # exp on the scalar engine leaves on the PE array
                            it += 1
                            if pending and it % stride == 0:
                                pending.pop(0)()
                            eab = epool.tile([128, 2 * NQ], F16, tag="eab")
                            sab3 = sab[:].rearrange("p (h q) -> p h q", h=2)
                            eab3 = eab[:].rearrange("p (h q) -> p h q", h=2)
                            nc.scalar.activation(
                                eab3[:, :, w0:NQ], sab3[:, :, w0:NQ],
                                AF.Exp, scale=0.125, bias=ebias[:])
                            if d >= 0:
                                nc.gpsimd.affine_select(
                                    out=eab3[:, :, w0:NQ],
                                    in_=eab3[:, :, w0:NQ],
                                    compare_op=mybir.AluOpType.is_ge,
                                    fill=0.0,
                                    base=0,
                                    channel_multiplier=-1,
                                    pattern=[[0, 2], [1, nw]],
                                )
                            prev = (kt, eab)
                        emit_y(*prev)
                        # normalize: yq rows = numerator / Z
                        zcA = zsbpool.tile([1, NQ], F32, tag="zc", name=f"zcA_{qc}_{pj}")
                        zcB = zsbpool.tile([1, NQ], F32, tag="zc", name=f"zcB_{qc}_{pj}")
                        nc.vector.tensor_copy(zcA[0:1, :], yaugA[64:65, :])
                        nc.vector.tensor_copy(zcB[0:1, :], yaugB[64:65, :])
                        ziA = zsbpool.tile([1, NQ], F32, tag="zi", name=f"ziA_{qc}_{pj}")
                        ziB = zsbpool.tile([1, NQ], F32, tag="zi", name=f"ziB_{qc}_{pj}")
                        nc.vector.reciprocal_approx_fast(out=ziA[0:1, :], in_=zcA[0:1, :])
                        nc.vector.reciprocal_approx_fast(out=ziB[0:1, :], in_=zcB[0:1, :])
                        zbA = zsbpool.tile([64, NQ], F32, tag="zbA", name=f"zbA_{qc}_{pj}")
                        zbB = zsbpool.tile([64, NQ], F32, tag="zbB", name=f"zbB_{qc}_{pj}")
                        nc.gpsimd.partition_broadcast(zbA[0:64, :], ziA[0:1, :], channels=64)
                        nc.gpsimd.partition_broadcast(zbB[0:64, :], ziB[0:1, :], channels=64)
                        nc.vector.tensor_mul(yq[pj][0:64, :], yaugA[0:64, :], zbA[0:64, :])
                        nc.vector.tensor_mul(yq[pj][64:128, :], yaugB[0:64, :], zbB[0:64, :])
                        # rope the next chunk's tiles here so they are ready
                        # long before the next q-chunk's attention starts
                        if qc + 1 < NCH:
                            if pj == 0:
                                emit_rope_tile(kt_sb, qc + 1, 0)
                                emit_rope_tile(qt[0], qc + 1, 1)
                            else:
                                emit_rope_tile(qt[pj], qc + 1, 1 + pj)
                    # o-proj for this q chunk: enqueue; drained inside the next
                    # chunk's inner loop
                    for m in range(16):
                        pending.append(
                            lambda qc=qc, yq=yq, m=m: emit_oproj_tile(qc, yq, m))
                tail_mode["on"] = True
                while pending:
                    pending.pop(0)()
            vtpool.release()
            xpool.release()
    nc.finalize()
    return nc


def _rope_tables():
    inv = 1.0 / (ROPE_BASE ** (np.arange(0, D, 2, dtype=np.float32) / D))
    fr_ = np.arange(T, dtype=np.float32)[:, None] * inv[None, :]
    cosT = np.cos(fr_).T.astype(np.float32)
    sinT = np.sin(fr_).T.astype(np.float32)
    cosfull = np.ascontiguousarray(np.tile(cosT, (4, 1)))
    sinfull = np.ascontiguousarray(np.concatenate([-sinT, sinT, -sinT, sinT]))
    return cosfull, sinfull


def _perm_matrix():
    p = np.zeros((128, 128), dtype=np.float32)
    for i in range(128):
        j = i + 32 if (i % 64) < 32 else i - 32
        p[i, j] = 1.0
    return p


def _get_nc():
    if "nc" not in _CACHE:
        _CACHE["nc"] = _build_nc()
    return _CACHE["nc"]


def make_in_maps(x, Wq, Wk, Wv, Wo):
    cosfull, sinfull = _rope_tables()
    permm = _perm_matrix()
    in_maps = []
    for c in range(8):
        b, r = divmod(c, 4)
        qcols = np.concatenate(
            [np.arange(64 * (8 * r + h), 64 * (8 * r + h) + 64) for h in LPERM])
        in_maps.append({
            "xT": np.ascontiguousarray(x[b].T).astype(np.float16),
            "wq": np.ascontiguousarray(Wq[:, qcols]).astype(np.float16),
            "wk": np.ascontiguousarray(Wk[:, 128 * r:128 * (r + 1)]).astype(np.float16),
            "wv": np.ascontiguousarray(Wv[:, 128 * r:128 * (r + 1)]).astype(np.float16),
            "wo": np.ascontiguousarray(Wo[qcols, :]).astype(np.float16),
            "cosf": cosfull.astype(np.float16),
            "sinf": sinfull.astype(np.float16),
            "perm": permm.astype(np.float16),
        })
    return in_maps


def run(x, Wq, Wk, Wv, Wo, **spmd_kwargs):
    from concourse.bass_utils import run_bass_kernel_spmd

    nc = _get_nc()
    in_maps = make_in_maps(x, Wq, Wk, Wv, Wo)
    res = run_bass_kernel_spmd(nc, in_maps, list(range(8)), **spmd_kwargs)
    out = np.zeros((B, T, C), dtype=np.float32)
    for c in range(8):
        out[c // 4] += res.results[c]["outT"].T.astype(np.float32)
    return out, res


def kernel(**inputs):
    x = np.asarray(inputs["x"], dtype=np.float32)
    Wq = np.asarray(inputs["Wq"], dtype=np.float32)
    Wk = np.asarray(inputs["Wk"], dtype=np.float32)
    Wv = np.asarray(inputs["Wv"], dtype=np.float32)
    Wo = np.asarray(inputs["Wo"], dtype=np.float32)
    out, _ = run(x, Wq, Wk, Wv, Wo)
    return out
